# revision 16
# baseline (speedup 1.0000x reference)
"""Trainium2 Bass kernel for strictly-causal RoPE self-attention (no softmax).

  out[b,h] = tril(rope(Q)@rope(Q)^T, -1) @ V    with K = Q.

Sharding: B*H = 8 independent (b,h) slices -> one per NeuronCore (pure data
parallel, no collectives). Per core: T=N=2048.

Per-core algorithm (device compute in bf16 matmul / f32 accumulate):
  - Host passes Q pre-transposed+deinterleaved (layout prep only), with the
    even/odd planes and the sin/cos tables CONCATENATED column-wise so each
    RoPE pair needs just two 1 MB DMAs (the in-order Sync queue serializes
    DMA issues, so issue count is precious):
      qteo[n', 0:T] = Q[t,2n'], qteo[n', T:2T] = Q[t,2n'+1]    [N/2, 2T]
      cst [n', 0:T] = sin,      cst [n', T:2T] = cos           [N/2, 2T]
  - Device RoPE per pair p, all six tensor_tensor ops on DVE (bf16 2x,
    ~1.21us/op -> ~7.3us/pair paces the fill; GpSimd tensor ops are NOT
    used during the fill - they contend for SBUF ports and stretch DVE ops
    ~4x):  t_es=te*st ; t_os=to*st ; te*=ct ; te-=t_os (QRT_E) ;
           to*=ct ; to+=t_es (QRT_O).
  - Stage 1 (PE): P[s,t] = sum_n QRT[n,s]*QRT[n,t], lower-triangle chains
    packed into 512-col PSUM "units" (34 units).  The kk contraction is
    emitted pair-major and SEGMENTED so the PE keeps working while RoPE
    streams: units 0-3 (banks 0-3, "pp") split A/B at pair 4 and advance
    with the stream; units 4-11 (banks 4-7, "pc") split s1/s2/B at pairs
    2/4 so s1 runs as soon as pairs 0-1 are resident and s2 after pairs
    2-3; units 12-15 split A/B; units 16-19 split at pair 6 (their first
    12 contraction steps fill the late-pair windows); units 20-33 run
    unsplit post-fill.  Segment partials evict to SBUF bf16 (ScalarE);
    s1+s2 partials are recombined by SWDGE DMA-accumulate (touches neither
    DVE nor PE); remaining folds + the strict-causal diagonal masks run on
    GpSimd after the fill.  In a packed unit only the first chain's first
    matmul sets start=True (start clears has_written for the WHOLE bank).
  - Stage 2 (PE): out[t,n] += P[s,t]^T @ V[s,n], accumulating j in PSUM
    ("pp" banks, free after the fill), evict on ScalarE, DMA out.
  - V loads are deferred until after the fill DMA stream.
"""

import os
import sys
import math

for _p in ("/opt/trn_rl_repo", "/root/.axon_site/_ro/trn_rl_repo"):
    if os.path.isdir(_p) and _p not in sys.path:
        sys.path.append(_p)

import numpy as np
import ml_dtypes

B, H, T, N = 2, 4, 2048, 2048
THETA = 2.0 ** 16
NCORES = 8
CW = 512  # superstep width (t-columns) and output n-chunk width

bf16 = ml_dtypes.bfloat16

LAST_RESULT = None  # BassKernelResults of the most recent run (for test.py)


def build_bass(t_len=T, n_dim=N, num_devices=NCORES):
    from concourse import bacc, mybir, tile

    nc = bacc.Bacc("TRN2", target_bir_lowering=False, debug=False,
                   num_devices=num_devices)
    bf = mybir.dt.bfloat16
    f32 = mybir.dt.float32
    mult = mybir.AluOpType.mult
    add = mybir.AluOpType.add

    nh = n_dim // 2
    npair = nh // 128          # RoPE pairs (8); pair p yields qrt[p], qrt[8+p]
    kk_n = n_dim // 128        # contraction tiles (16)
    nb = t_len // 128          # t-blocks (16)
    ncks = t_len // CW         # supersteps (4)
    sw = CW // 128             # t-blocks per superstep (4)
    nch = n_dim // CW          # output n-chunks (4)

    # kk emission order: pair-major so each RoPE pair unlocks two steps.
    kkseq = []
    for p in range(npair):
        kkseq += [p, npair + p]

    qteo = nc.declare_dram_parameter("qteo", [nh, 2 * t_len], bf,
                                     isOutput=False)
    cstd = nc.declare_dram_parameter("cst", [nh, 2 * t_len], bf,
                                     isOutput=False)
    vin = nc.declare_dram_parameter("v", [t_len, n_dim], bf, isOutput=False)
    maskd = nc.declare_dram_parameter("mask", [128, 128], bf, isOutput=False)
    outd = nc.declare_dram_parameter("out", [t_len, n_dim], f32, isOutput=True)

    # ---- stage-1 chain/unit bookkeeping ----
    def chain(ic, j):
        rj0 = max(128 * j, CW * ic)
        return {"ic": ic, "j": j, "rj0": rj0, "w": CW * (ic + 1) - rj0,
                "off": 0}

    ch = {(ic, j): chain(ic, j) for ic in range(ncks)
          for j in range(sw * ic + sw)}

    units = []

    def unit(*keys):
        cs, off = [], 0
        for k in keys:
            c = dict(ch[k])
            c["off"] = off
            off += c["w"]
            cs.append(c)
        assert off <= CW
        units.append({"chains": cs, "w": off, "ps": None, "parts": [],
                      "seg": None, "tag": None, "uid": len(units)})

    unit((0, 0))
    unit((0, 1), (0, 3))
    unit((0, 2), (1, 6))
    unit((1, 0))
    unit((1, 1))
    unit((1, 2))
    unit((1, 3))
    unit((1, 4))
    unit((1, 5), (1, 7))
    for j in range(9):
        unit((2, j))
    unit((2, 9), (2, 11))
    unit((2, 10), (3, 14))
    for j in range(13):
        unit((3, j))
    unit((3, 13), (3, 15))
    assert sum(u["w"] for u in units) == 17408 and len(units) == 34

    SEG2 = [(0, 8), (8, 16)]                    # A (pairs 0-3) / B (4-7)
    SEG3 = [(0, 4), (4, 8), (8, 16)]            # s1 / s2 / B
    SEGL = [(0, 12), (12, 16)]                  # A' (pairs 0-5) / B'
    SEG1 = [(0, 16)]
    # partial-eviction tag per segment class (sized by lifetime)
    for i, u in enumerate(units):
        if i < 3:
            u["seg"], u["tag"] = SEG2, "pp"
        elif i < 12:
            u["seg"], u["tag"] = SEG3, "pc"
        elif i < 16:
            u["seg"], u["tag"] = SEG2, "pc"
        elif i < 20:
            u["seg"], u["tag"] = SEGL, "pc"
        else:
            u["seg"], u["tag"] = SEG1, "pc"

    ptiles = {ic: {} for ic in range(ncks)}  # ic -> j -> (tile, rj0, off)

    with tile.TileContext(nc) as tc:
        with (
            tc.tile_pool(name="qrt", bufs=npair) as qrt_pool,
            tc.tile_pool(name="vres", bufs=nb) as v_pool,
            tc.tile_pool(name="tbl", bufs=2) as tbl_pool,
            tc.tile_pool(name="rtmp", bufs=2) as tmp_pool,
            tc.tile_pool(name="part", bufs=8) as part_pool,
            tc.tile_pool(name="osb", bufs=2) as out_pool,
            tc.tile_pool(name="mk", bufs=1) as mk_pool,
            tc.tile_pool(name="warm", bufs=1) as warm_pool,
            tc.tile_pool(name="psumW", bufs=1, space="PSUM") as pw_pool,
            tc.tile_pool(name="psumP", bufs=3, space="PSUM") as pp_pool,
            tc.tile_pool(name="psumC", bufs=4, space="PSUM") as pc_pool,
        ):
            # Warm-up scratch: GpSimd memsets it at t~0; dummy matmuls on it
            # keep the PE HAM clock-gate at 8/8 until real work arrives.
            scratch = warm_pool.tile([128, 256], bf)
            nc.gpsimd.memset(scratch[:], 0.0)
            mask_sb = mk_pool.tile([128, 128], bf)

            qrt = [None] * kk_n
            qtiles, ctiles, ttiles = {}, {}, {}

            def emit_pair(p):
                qp = qrt_pool.tile([128, 2 * t_len], bf, tag="qrt",
                                   name=f"qp_{p}")
                cs = tbl_pool.tile([128, 2 * t_len], bf, tag="tbl",
                                   name=f"cs_{p}")
                rr = slice(128 * p, 128 * p + 128)
                te, to = qp[:, 0:t_len], qp[:, t_len:2 * t_len]
                st, ct = cs[:, 0:t_len], cs[:, t_len:2 * t_len]
                hl = t_len // 2
                if p == 0:
                    # column-split pair 0's DMA so the left t-half lands first
                    for h0 in (0, hl):
                        hs = slice(h0, h0 + hl)
                        nc.sync.dma_start(cs[:, hs], cstd[rr, hs])
                        nc.sync.dma_start(qp[:, hs], qteo[rr, hs])
                        ths = slice(t_len + h0, t_len + h0 + hl)
                        nc.sync.dma_start(qp[:, ths], qteo[rr, ths])
                        nc.sync.dma_start(cs[:, ths], cstd[rr, ths])
                else:
                    nc.sync.dma_start(qp[:], qteo[rr, :])
                    nc.sync.dma_start(cs[:], cstd[rr, :])
                t_es = tmp_pool.tile([128, t_len], bf, tag="tmp",
                                     name=f"tes_{p}")
                t_os = tmp_pool.tile([128, t_len], bf, tag="tmp",
                                     name=f"tos_{p}")
                halves = (slice(0, hl), slice(hl, t_len)) if p < 2 \
                    else (slice(0, t_len),)
                for hs in halves:
                    nc.vector.tensor_mul(t_es[:, hs], te[:, hs], st[:, hs])
                    nc.vector.tensor_mul(t_os[:, hs], to[:, hs], st[:, hs])
                    nc.vector.tensor_mul(te[:, hs], te[:, hs], ct[:, hs])
                    nc.vector.tensor_mul(to[:, hs], to[:, hs], ct[:, hs])
                    nc.vector.tensor_sub(te[:, hs], te[:, hs], t_os[:, hs])
                    nc.vector.tensor_add(to[:, hs], to[:, hs], t_es[:, hs])
                qrt[p] = te
                qrt[npair + p] = to
                qtiles[p], ctiles[p], ttiles[p] = qp, cs, t_es

            # ---- stage-1 machinery ----
            def seg_of(u, kki):
                for s in u["seg"]:
                    if s[0] <= kki < s[1]:
                        return s
                raise AssertionError

            def part_tag(u, s1):
                if s1 == 16:
                    return "pfin", 22
                if u["seg"] is SEG3:
                    return ("ps1", 9) if s1 == 4 else ("ps2", 9)
                return "pA", 11

            def mm_step(u, kki):
                s0, s1 = seg_of(u, kki)
                if kki == s0:
                    pool = pp_pool if u["tag"] == "pp" else pc_pool
                    u["ps"] = pool.tile([128, u["w"]], f32, tag=u["tag"],
                                        name=f"ps_u{u['uid']}_{s0}")
                kk = kkseq[kki]
                for ci, c in enumerate(u["chains"]):
                    # start=True clears has_written for the WHOLE bank, so
                    # only the first chain of a packed unit may set it; the
                    # second chain's first matmul overwrites-where-unset.
                    nc.tensor.matmul(
                        u["ps"][:, c["off"]:c["off"] + c["w"]],
                        qrt[kk][:, 128 * c["j"]:128 * c["j"] + 128],
                        qrt[kk][:, c["rj0"]:c["rj0"] + c["w"]],
                        start=(kki == s0 and ci == 0),
                        stop=(kki == s1 - 1))
                if kki == s1 - 1:
                    tag, bufs = part_tag(u, s1)
                    part = part_pool.tile([128, u["w"]], bf, tag=tag,
                                          bufs=bufs,
                                          name=f"pa_u{u['uid']}_{s0}")
                    nc.scalar.copy(part[:, :], u["ps"][:, :])
                    u["parts"].append(part)
                    u["ps"] = None

            def mm_full(u, k0, k1):
                for kki in range(k0, k1):
                    mm_step(u, kki)

            def finalize(u):
                parts = [x for x in u["parts"] if x is not None]
                pt = parts[-1]
                for x in parts[:-1]:
                    nc.gpsimd.tensor_tensor(pt[:, :], pt[:, :], x[:, :], add)
                for c in u["chains"]:
                    if c["rj0"] == 128 * c["j"]:
                        o = c["off"]
                        nc.gpsimd.tensor_tensor(pt[:, o:o + 128],
                                                pt[:, o:o + 128],
                                                mask_sb[:], mult)
                    ptiles[c["ic"]][c["j"]] = (pt, c["rj0"], c["off"])

            # ---- stage 2 ----
            v_tiles = [None] * nb

            def stage2(ic):
                pmap = ptiles[ic]
                for d in range(sw):
                    i = sw * ic + d
                    ti = 128 * i
                    for chk in range(nch):
                        ops = pp_pool.tile([128, CW], f32, tag="pp",
                                           name=f"ps2_{i}_{chk}")
                        for j in range(i + 1):
                            pt, rj0, off = pmap[j]
                            o = off + ti - rj0
                            nc.tensor.matmul(
                                ops[:, :], pt[:, o:o + 128],
                                v_tiles[j][:, CW * chk:CW * (chk + 1)],
                                start=(j == 0), stop=(j == i))
                        osb = out_pool.tile([128, CW], f32, tag="osb",
                                            name=f"osb_{i}_{chk}")
                        nc.scalar.copy(osb[:], ops[:])
                        nc.sync.dma_start(
                            outd[ti:ti + 128, CW * chk:CW * (chk + 1)],
                            osb[:])

            # ---- emission schedule ----
            emit_pair(0)
            emit_pair(1)
            # mask DMA off the critical issue path (needed only at finalize)
            nc.sync.dma_start(mask_sb[:], maskd[:])
            for p in range(2, npair):
                emit_pair(p)
            # Warm-up dummies on a dedicated PSUM bank: keep the HAM
            # clock-gate at 8/8 through the fill.  Anchored blocks read a
            # tile that lands mid-fill, so the in-order PE queue self-paces
            # them into predicted idle windows.
            ps_d = pw_pool.tile([128, 256], f32, tag="pw", name="ps_warm")

            def dummies(n, anchor=None):
                a = scratch if anchor is None else anchor
                for _ in range(n):
                    nc.tensor.matmul(ps_d[:, :], a[:, 0:128], a[:, 0:256],
                                     start=True, stop=True)

            dummies(48)
            # A: paced window, pairs 0-1: units 0-6 advance with the stream.
            for kki in range(0, 2):
                for u in units[0:7]:
                    mm_step(u, kki)
            dummies(6, qtiles[1])
            dummies(6, qtiles[2])
            dummies(6, qtiles[3])
            for kki in range(2, 4):
                for u in units[0:7]:
                    mm_step(u, kki)
            dummies(6, ctiles[2])
            # B: units 7-11 s1 full-speed on resident pairs 0-1.
            for u in units[7:12]:
                mm_full(u, 0, 4)
            dummies(6, ctiles[3])
            # C: pairs 2-3: units 0-2 continue A, units 3-6 s2.
            for kki in range(4, 8):
                for u in units[0:7]:
                    mm_step(u, kki)
            dummies(6, ctiles[4])
            # D: units 7-11 s2 full-speed (s1+s2 fold happens in finalize,
            # on GpSimd, after the fill - GpSimd tensor ops during the fill
            # contend with DVE SBUF ports, and SWDGE DMA-accum serializes
            # into the fill-DMA completion lanes).
            for u in units[7:12]:
                mm_full(u, 4, 8)
            dummies(6, ctiles[5])
            # E: units 12-15 A full-speed on resident pairs 0-3.
            for u in units[12:16]:
                mm_full(u, 0, 8)
            # F/G/H/I/J: pairs 4-7 windows: units 0-2 B paced, units 16-19
            # A' (12 steps) filling the gaps.
            for kki in range(8, 10):
                for u in units[0:3]:
                    mm_step(u, kki)
            dummies(6, ctiles[6])
            for u in units[16:20]:
                mm_full(u, 0, 12)
            for kki in range(10, 12):
                for u in units[0:3]:
                    mm_step(u, kki)
            for u in units[3:7]:
                mm_full(u, 8, 12)
            dummies(6, ctiles[7])
            for kki in range(12, 14):
                for u in units[0:7]:
                    mm_step(u, kki)
            dummies(6, ttiles[7])
            for kki in range(14, 16):
                for u in units[0:7]:
                    mm_step(u, kki)
            for u in units[0:7]:
                finalize(u)
            # V loads: issue after the whole fill DMA stream.
            for jb in range(nb):
                vt = v_pool.tile([128, n_dim], bf, tag="vt",
                                 name=f"vt_{jb}")
                nc.sync.dma_start(vt[:], vin[128 * jb:128 * (jb + 1), :])
                v_tiles[jb] = vt
            # F4: B segments + unsplit chains, interleaved with stage 2.
            # stage2(1) only needs ic1 ptiles (units 2-8), so it runs right
            # after stage2(0) while GpSimd folds units 9-19 in parallel.
            for u in units[7:9]:
                mm_full(u, 8, 16)
                finalize(u)
            stage2(0)
            stage2(1)
            for u in units[9:16]:
                mm_full(u, 8, 16)
                finalize(u)
            for u in units[16:20]:
                mm_full(u, 12, 16)
                finalize(u)
            for u in units[20:27]:
                mm_full(u, 0, 16)
                finalize(u)
            stage2(2)
            for u in units[27:34]:
                mm_full(u, 0, 16)
                finalize(u)
            stage2(3)

    nc.compile()
    return nc


def _tables(t_len=T, n_dim=N):
    t = np.arange(n_dim, dtype=np.float32)
    q = np.floor(t / 2.0) * 2.0
    f = (1.0 / THETA ** (q.astype(np.float64) / n_dim)
         / (2.0 * math.pi)).astype(np.float32)
    phases = np.arange(t_len, dtype=np.float32)[:, None] * f[None, :]
    ph = (phases % 1.0) * np.float32(2.0 * math.pi)
    ct = np.ascontiguousarray(np.cos(ph)[:, 0::2].T)  # [N/2, T]
    st = np.ascontiguousarray(np.sin(ph)[:, 0::2].T)
    return np.concatenate([st, ct], axis=1).astype(bf16)  # [N/2, 2T]


def _mask128():
    s = np.arange(128)[:, None]
    tt = np.arange(128)[None, :]
    return (s < tt).astype(bf16)


_compiled = {}


def _get_nc():
    if "nc" not in _compiled:
        _compiled["nc"] = build_bass()
    return _compiled["nc"]


def kernel(Q, V):
    global LAST_RESULT
    from concourse.bass_utils import run_bass_kernel_spmd

    Q = np.asarray(Q)
    V = np.asarray(V)
    assert Q.shape == (B, H, T, N) and V.shape == (B, H, T, N)

    nc = _get_nc()
    cst = _tables()
    mask = _mask128()

    in_maps = []
    for b in range(B):
        for h in range(H):
            qs = Q[b, h]
            qteo = np.concatenate(
                [qs[:, 0::2].T, qs[:, 1::2].T], axis=1)  # [N/2, 2T]
            in_maps.append({
                "qteo": np.ascontiguousarray(qteo).astype(bf16),
                "cst": cst,
                "v": V[b, h].astype(bf16),
                "mask": mask,
            })

    res = run_bass_kernel_spmd(nc, in_maps, core_ids=list(range(NCORES)))
    LAST_RESULT = res

    out = np.empty((B, H, T, N), dtype=np.float32)
    for b in range(B):
        for h in range(H):
            out[b, h] = res.results[b * H + h]["out"]
    return out


# revision 20
# speedup vs baseline: 1.1526x; 1.1526x over previous
"""Trainium2 Bass kernel for strictly-causal RoPE self-attention (no softmax).

  out[b,h] = tril(rope(Q)@rope(Q)^T, -1) @ V    with K = Q.

Sharding: B*H = 8 independent (b,h) slices -> one per NeuronCore (pure data
parallel, no collectives). Per core: T=N=2048.

Per-core algorithm (device compute in bf16 matmul / f32 accumulate):
  - Host passes Q pre-transposed+deinterleaved (layout prep only), with the
    even/odd planes and the sin/cos tables CONCATENATED column-wise so each
    RoPE pair needs just two 1 MB DMAs (the in-order Sync queue serializes
    DMA issues, so issue count is precious):
      qteo[n', 0:T] = Q[t,2n'], qteo[n', T:2T] = Q[t,2n'+1]    [N/2, 2T]
      cst [n', 0:T] = sin,      cst [n', T:2T] = cos           [N/2, 2T]
  - Device RoPE per pair p, all six tensor_tensor ops on DVE (bf16 2x,
    ~1.21us/op -> ~7.3us/pair paces the fill; GpSimd tensor ops are NOT
    used during the fill - they contend for SBUF ports and stretch DVE ops
    ~4x):  t_es=te*st ; t_os=to*st ; te*=ct ; te-=t_os (QRT_E) ;
           to*=ct ; to+=t_es (QRT_O).
  - Stage 1 (PE): P[s,t] = sum_n QRT[n,s]*QRT[n,t], lower-triangle chains
    packed into 512-col PSUM "units" (34 units).  The kk contraction is
    emitted pair-major and SEGMENTED so the PE keeps working while RoPE
    streams: units 0-3 (banks 0-3, "pp") split A/B at pair 4 and advance
    with the stream; units 4-11 (banks 4-7, "pc") split s1/s2/B at pairs
    2/4 so s1 runs as soon as pairs 0-1 are resident and s2 after pairs
    2-3; units 12-15 split A/B; units 16-19 split at pair 6 (their first
    12 contraction steps fill the late-pair windows); units 20-33 run
    unsplit post-fill.  Segment partials evict to SBUF bf16 (ScalarE);
    s1+s2 partials are recombined by SWDGE DMA-accumulate (touches neither
    DVE nor PE); remaining folds + the strict-causal diagonal masks run on
    GpSimd after the fill.  In a packed unit only the first chain's first
    matmul sets start=True (start clears has_written for the WHOLE bank).
  - Stage 2 (PE): out[t,n] += P[s,t]^T @ V[s,n], accumulating j in PSUM
    ("pp" banks, free after the fill), evict on ScalarE, DMA out.
  - V loads are deferred until after the fill DMA stream.
"""

import os
import sys
import math

for _p in ("/opt/trn_rl_repo", "/root/.axon_site/_ro/trn_rl_repo"):
    if os.path.isdir(_p) and _p not in sys.path:
        sys.path.append(_p)

import numpy as np
import ml_dtypes

B, H, T, N = 2, 4, 2048, 2048
THETA = 2.0 ** 16
NCORES = 8
CW = 512  # superstep width (t-columns) and output n-chunk width

bf16 = ml_dtypes.bfloat16

LAST_RESULT = None  # BassKernelResults of the most recent run (for test.py)


def build_bass(t_len=T, n_dim=N, num_devices=NCORES):
    from concourse import bacc, mybir, tile

    nc = bacc.Bacc("TRN2", target_bir_lowering=False, debug=False,
                   num_devices=num_devices)
    bf = mybir.dt.bfloat16
    f32 = mybir.dt.float32
    mult = mybir.AluOpType.mult
    add = mybir.AluOpType.add

    nh = n_dim // 2
    npair = nh // 128          # RoPE pairs (8); pair p yields qrt[p], qrt[8+p]
    kk_n = n_dim // 128        # contraction tiles (16)
    nb = t_len // 128          # t-blocks (16)
    ncks = t_len // CW         # supersteps (4)
    sw = CW // 128             # t-blocks per superstep (4)
    nch = n_dim // CW          # output n-chunks (4)

    # kk emission order: pair-major so each RoPE pair unlocks two steps.
    kkseq = []
    for p in range(npair):
        kkseq += [p, npair + p]

    qteo = nc.declare_dram_parameter("qteo", [nh, 2 * t_len], bf,
                                     isOutput=False)
    cstd = nc.declare_dram_parameter("cst", [nh, 2 * t_len], bf,
                                     isOutput=False)
    vin = nc.declare_dram_parameter("v", [t_len, n_dim], bf, isOutput=False)
    maskd = nc.declare_dram_parameter("mask", [128, 128], bf, isOutput=False)
    outd = nc.declare_dram_parameter("out", [t_len, n_dim], f32, isOutput=True)

    # ---- stage-1 chain/unit bookkeeping ----
    def chain(ic, j):
        rj0 = max(128 * j, CW * ic)
        return {"ic": ic, "j": j, "rj0": rj0, "w": CW * (ic + 1) - rj0,
                "off": 0}

    ch = {(ic, j): chain(ic, j) for ic in range(ncks)
          for j in range(sw * ic + sw)}

    units = []

    def unit(*keys):
        cs, off = [], 0
        for k in keys:
            c = dict(ch[k])
            c["off"] = off
            off += c["w"]
            cs.append(c)
        assert off <= CW
        units.append({"chains": cs, "w": off, "ps": None, "parts": [],
                      "seg": None, "tag": None, "uid": len(units)})

    unit((0, 0))
    unit((0, 1), (0, 3))
    unit((0, 2), (1, 6))
    unit((1, 0))
    unit((1, 1))
    unit((1, 2))
    unit((1, 3))
    unit((1, 4))
    unit((1, 5), (1, 7))
    for j in range(9):
        unit((2, j))
    unit((2, 9), (2, 11))
    unit((2, 10), (3, 14))
    for j in range(13):
        unit((3, j))
    unit((3, 13), (3, 15))
    assert sum(u["w"] for u in units) == 17408 and len(units) == 34

    SEG2 = [(0, 8), (8, 16)]                    # A (pairs 0-3) / B (4-7)
    SEG3 = [(0, 4), (4, 8), (8, 16)]            # s1 / s2 / B
    SEGL = [(0, 12), (12, 16)]                  # A' (pairs 0-5) / B'
    SEG1 = [(0, 16)]
    # partial-eviction tag per segment class (sized by lifetime)
    for i, u in enumerate(units):
        if i < 3:
            u["seg"], u["tag"] = SEG2, "pp"
        elif i < 12:
            u["seg"], u["tag"] = SEG3, "pc"
        elif i < 16:
            u["seg"], u["tag"] = SEG2, "pc"
        elif i < 20:
            u["seg"], u["tag"] = SEGL, "pc"
        else:
            u["seg"], u["tag"] = SEG1, "pc"

    ptiles = {ic: {} for ic in range(ncks)}  # ic -> j -> (tile, rj0, off)

    with tile.TileContext(nc) as tc:
        with (
            tc.tile_pool(name="qrt", bufs=npair) as qrt_pool,
            tc.tile_pool(name="vres", bufs=nb) as v_pool,
            tc.tile_pool(name="tbl", bufs=2) as tbl_pool,
            tc.tile_pool(name="rtmp", bufs=2) as tmp_pool,
            tc.tile_pool(name="part", bufs=8) as part_pool,
            tc.tile_pool(name="osb", bufs=2) as out_pool,
            tc.tile_pool(name="mk", bufs=1) as mk_pool,
            tc.tile_pool(name="warm", bufs=1) as warm_pool,
            tc.tile_pool(name="psumW", bufs=1, space="PSUM") as pw_pool,
            tc.tile_pool(name="psumP", bufs=3, space="PSUM") as pp_pool,
            tc.tile_pool(name="psumC", bufs=4, space="PSUM") as pc_pool,
        ):
            # Warm-up scratch: GpSimd memsets it at t~0; dummy matmuls on it
            # keep the PE HAM clock-gate at 8/8 until real work arrives.
            scratch = warm_pool.tile([128, 256], bf)
            nc.gpsimd.memset(scratch[:], 0.0)
            mask_sb = mk_pool.tile([128, 128], bf)

            qrt = [None] * kk_n
            qtiles, ctiles, ttiles = {}, {}, {}

            def emit_pair(p):
                qp = qrt_pool.tile([128, 2 * t_len], bf, tag="qrt",
                                   name=f"qp_{p}")
                cs = tbl_pool.tile([128, 2 * t_len], bf, tag="tbl",
                                   name=f"cs_{p}")
                rr = slice(128 * p, 128 * p + 128)
                te, to = qp[:, 0:t_len], qp[:, t_len:2 * t_len]
                st, ct = cs[:, 0:t_len], cs[:, t_len:2 * t_len]
                hl = t_len // 2
                if p == 0:
                    # column-split pair 0's DMA so the left t-half lands first
                    for h0 in (0, hl):
                        hs = slice(h0, h0 + hl)
                        nc.sync.dma_start(cs[:, hs], cstd[rr, hs])
                        nc.sync.dma_start(qp[:, hs], qteo[rr, hs])
                        ths = slice(t_len + h0, t_len + h0 + hl)
                        nc.sync.dma_start(qp[:, ths], qteo[rr, ths])
                        nc.sync.dma_start(cs[:, ths], cstd[rr, ths])
                else:
                    nc.sync.dma_start(qp[:], qteo[rr, :])
                    nc.sync.dma_start(cs[:], cstd[rr, :])
                t_es = tmp_pool.tile([128, t_len], bf, tag="tmp",
                                     name=f"tes_{p}")
                t_os = tmp_pool.tile([128, t_len], bf, tag="tmp",
                                     name=f"tos_{p}")
                halves = (slice(0, hl), slice(hl, t_len)) if p < 2 \
                    else (slice(0, t_len),)
                for hs in halves:
                    nc.vector.tensor_mul(t_es[:, hs], te[:, hs], st[:, hs])
                    nc.vector.tensor_mul(t_os[:, hs], to[:, hs], st[:, hs])
                    nc.vector.tensor_mul(te[:, hs], te[:, hs], ct[:, hs])
                    nc.vector.tensor_mul(to[:, hs], to[:, hs], ct[:, hs])
                    nc.vector.tensor_sub(te[:, hs], te[:, hs], t_os[:, hs])
                    nc.vector.tensor_add(to[:, hs], to[:, hs], t_es[:, hs])
                qrt[p] = te
                qrt[npair + p] = to
                qtiles[p], ctiles[p], ttiles[p] = qp, cs, t_es

            # ---- stage-1 machinery ----
            def seg_of(u, kki):
                for s in u["seg"]:
                    if s[0] <= kki < s[1]:
                        return s
                raise AssertionError

            def part_tag(u, s1):
                if s1 == 16:
                    return "pfin", 22
                if u["seg"] is SEG3:
                    return ("ps1", 9) if s1 == 4 else ("ps2", 9)
                return "pA", 11

            def mm_step(u, kki):
                s0, s1 = seg_of(u, kki)
                if kki == s0:
                    pool = pp_pool if u["tag"] == "pp" else pc_pool
                    u["ps"] = pool.tile([128, u["w"]], f32, tag=u["tag"],
                                        name=f"ps_u{u['uid']}_{s0}")
                kk = kkseq[kki]
                for ci, c in enumerate(u["chains"]):
                    # start=True clears has_written for the WHOLE bank, so
                    # only the first chain of a packed unit may set it; the
                    # second chain's first matmul overwrites-where-unset.
                    nc.tensor.matmul(
                        u["ps"][:, c["off"]:c["off"] + c["w"]],
                        qrt[kk][:, 128 * c["j"]:128 * c["j"] + 128],
                        qrt[kk][:, c["rj0"]:c["rj0"] + c["w"]],
                        start=(kki == s0 and ci == 0),
                        stop=(kki == s1 - 1))
                if kki == s1 - 1:
                    tag, bufs = part_tag(u, s1)
                    part = part_pool.tile([128, u["w"]], bf, tag=tag,
                                          bufs=bufs,
                                          name=f"pa_u{u['uid']}_{s0}")
                    nc.scalar.copy(part[:, :], u["ps"][:, :])
                    u["parts"].append(part)
                    u["ps"] = None

            def mm_full(u, k0, k1):
                for kki in range(k0, k1):
                    mm_step(u, kki)

            def finalize(u):
                # folds + masks on DVE: it is idle once the RoPE fill ends,
                # and its tensor_tensor is ~4x faster than GpSimd's.
                parts = [x for x in u["parts"] if x is not None]
                pt = parts[-1]
                for x in parts[:-1]:
                    nc.vector.tensor_add(pt[:, :], pt[:, :], x[:, :])
                for c in u["chains"]:
                    if c["rj0"] == 128 * c["j"]:
                        o = c["off"]
                        nc.vector.tensor_mul(pt[:, o:o + 128],
                                             pt[:, o:o + 128], mask_sb[:])
                    ptiles[c["ic"]][c["j"]] = (pt, c["rj0"], c["off"])

            # ---- stage 2 ----
            v_tiles = [None] * nb

            def stage2(ic):
                pmap = ptiles[ic]
                for d in range(sw):
                    i = sw * ic + d
                    ti = 128 * i
                    for chk in range(nch):
                        ops = pp_pool.tile([128, CW], f32, tag="pp",
                                           name=f"ps2_{i}_{chk}")
                        for j in range(i + 1):
                            pt, rj0, off = pmap[j]
                            o = off + ti - rj0
                            nc.tensor.matmul(
                                ops[:, :], pt[:, o:o + 128],
                                v_tiles[j][:, CW * chk:CW * (chk + 1)],
                                start=(j == 0), stop=(j == i))
                        osb = out_pool.tile([128, CW], f32, tag="osb",
                                            name=f"osb_{i}_{chk}")
                        nc.scalar.copy(osb[:], ops[:])
                        nc.sync.dma_start(
                            outd[ti:ti + 128, CW * chk:CW * (chk + 1)],
                            osb[:])

            # ---- emission schedule ----
            emit_pair(0)
            emit_pair(1)
            # mask DMA off the critical issue path (needed only at finalize)
            nc.sync.dma_start(mask_sb[:], maskd[:])
            for p in range(2, npair):
                emit_pair(p)
            # Warm-up dummies on a dedicated PSUM bank: keep the HAM
            # clock-gate at 8/8 through the fill.  Anchored blocks read a
            # tile that lands mid-fill, so the in-order PE queue self-paces
            # them into predicted idle windows.
            ps_d = pw_pool.tile([128, 256], f32, tag="pw", name="ps_warm")

            def dummies(n, anchor=None):
                a = scratch if anchor is None else anchor
                for _ in range(n):
                    nc.tensor.matmul(ps_d[:, :], a[:, 0:128], a[:, 0:256],
                                     start=True, stop=True)

            dummies(48)
            # A: paced window, pairs 0-1: units 0-6 advance with the stream.
            # Anchored dummy blocks may only read tiles from pools that
            # never recycle their slot (qp; partials with exact alloc
            # counts; the LAST pair's cs/tmp tiles) - a cross-engine reader
            # on a rotating slot deadlocks the in-order queues.
            for kki in range(0, 2):
                for u in units[0:7]:
                    mm_step(u, kki)
            dummies(6, qtiles[1])
            dummies(6, qtiles[2])
            dummies(6, qtiles[3])
            for kki in range(2, 4):
                for u in units[0:7]:
                    mm_step(u, kki)
            # B: units 7-11 s1 full-speed on resident pairs 0-1.
            for u in units[7:12]:
                mm_full(u, 0, 4)
            dummies(6, units[3]["parts"][0])
            dummies(6, units[7]["parts"][0])
            # C: pairs 2-3: units 0-2 continue A, units 3-6 s2.
            for kki in range(4, 6):
                for u in units[0:7]:
                    mm_step(u, kki)
            dummies(6, units[9]["parts"][0])
            dummies(6, units[11]["parts"][0])
            for kki in range(6, 8):
                for u in units[0:7]:
                    mm_step(u, kki)
            dummies(6, units[3]["parts"][1])
            # D: units 7-11 s2 full-speed (s1+s2 fold happens in finalize,
            # on the post-fill-idle DVE).
            for u in units[7:12]:
                mm_full(u, 4, 8)
            dummies(6, units[7]["parts"][1])
            dummies(6, units[11]["parts"][1])
            # E: units 12-15 A full-speed on resident pairs 0-3.
            for u in units[12:16]:
                mm_full(u, 0, 8)
            dummies(6, units[12]["parts"][0])
            dummies(6, units[15]["parts"][0])
            # F/G/H/I/J: pairs 4-7 windows: units 0-2 B paced, units 16-19
            # A' (12 steps) filling the gaps.
            for kki in range(8, 10):
                for u in units[0:3]:
                    mm_step(u, kki)
            for u in units[16:20]:
                mm_full(u, 0, 12)
            for kki in range(10, 12):
                for u in units[0:3]:
                    mm_step(u, kki)
            for u in units[3:7]:
                mm_full(u, 8, 12)
            dummies(6, ctiles[7])
            for kki in range(12, 14):
                for u in units[0:7]:
                    mm_step(u, kki)
            dummies(6, ttiles[7])
            for kki in range(14, 16):
                for u in units[0:7]:
                    mm_step(u, kki)
            for u in units[0:7]:
                finalize(u)
            # V loads: issue after the whole fill DMA stream.
            for jb in range(nb):
                vt = v_pool.tile([128, n_dim], bf, tag="vt",
                                 name=f"vt_{jb}")
                nc.sync.dma_start(vt[:], vin[128 * jb:128 * (jb + 1), :])
                v_tiles[jb] = vt
            # F4: B segments + unsplit chains, interleaved with stage 2.
            # stage2(1) only needs ic1 ptiles (units 2-8), so it runs right
            # after stage2(0) while GpSimd folds units 9-19 in parallel.
            for u in units[7:9]:
                mm_full(u, 8, 16)
                finalize(u)
            stage2(0)
            stage2(1)
            for u in units[9:16]:
                mm_full(u, 8, 16)
                finalize(u)
            for u in units[16:20]:
                mm_full(u, 12, 16)
                finalize(u)
            for u in units[20:27]:
                mm_full(u, 0, 16)
                finalize(u)
            stage2(2)
            for u in units[27:34]:
                mm_full(u, 0, 16)
                finalize(u)
            stage2(3)

    nc.compile()
    return nc


def _tables(t_len=T, n_dim=N):
    t = np.arange(n_dim, dtype=np.float32)
    q = np.floor(t / 2.0) * 2.0
    f = (1.0 / THETA ** (q.astype(np.float64) / n_dim)
         / (2.0 * math.pi)).astype(np.float32)
    phases = np.arange(t_len, dtype=np.float32)[:, None] * f[None, :]
    ph = (phases % 1.0) * np.float32(2.0 * math.pi)
    ct = np.ascontiguousarray(np.cos(ph)[:, 0::2].T)  # [N/2, T]
    st = np.ascontiguousarray(np.sin(ph)[:, 0::2].T)
    return np.concatenate([st, ct], axis=1).astype(bf16)  # [N/2, 2T]


def _mask128():
    s = np.arange(128)[:, None]
    tt = np.arange(128)[None, :]
    return (s < tt).astype(bf16)


_compiled = {}


def _get_nc():
    if "nc" not in _compiled:
        _compiled["nc"] = build_bass()
    return _compiled["nc"]


def kernel(Q, V):
    global LAST_RESULT
    from concourse.bass_utils import run_bass_kernel_spmd

    Q = np.asarray(Q)
    V = np.asarray(V)
    assert Q.shape == (B, H, T, N) and V.shape == (B, H, T, N)

    nc = _get_nc()
    cst = _tables()
    mask = _mask128()

    in_maps = []
    for b in range(B):
        for h in range(H):
            qs = Q[b, h]
            qteo = np.concatenate(
                [qs[:, 0::2].T, qs[:, 1::2].T], axis=1)  # [N/2, 2T]
            in_maps.append({
                "qteo": np.ascontiguousarray(qteo).astype(bf16),
                "cst": cst,
                "v": V[b, h].astype(bf16),
                "mask": mask,
            })

    res = run_bass_kernel_spmd(nc, in_maps, core_ids=list(range(NCORES)))
    LAST_RESULT = res

    out = np.empty((B, H, T, N), dtype=np.float32)
    for b in range(B):
        for h in range(H):
            out[b, h] = res.results[b * H + h]["out"]
    return out


# revision 21
# speedup vs baseline: 1.1898x; 1.0322x over previous
"""Trainium2 Bass kernel for strictly-causal RoPE self-attention (no softmax).

  out[b,h] = tril(rope(Q)@rope(Q)^T, -1) @ V    with K = Q.

Sharding: B*H = 8 independent (b,h) slices -> one per NeuronCore (pure data
parallel, no collectives). Per core: T=N=2048.

Per-core algorithm (device compute in bf16 matmul / f32 accumulate):
  - Host passes Q pre-transposed+deinterleaved (layout prep only), with the
    even/odd planes and the sin/cos tables CONCATENATED column-wise so each
    RoPE pair needs just two 1 MB DMAs (the in-order Sync queue serializes
    DMA issues, so issue count is precious):
      qteo[n', 0:T] = Q[t,2n'], qteo[n', T:2T] = Q[t,2n'+1]    [N/2, 2T]
      cst [n', 0:T] = sin,      cst [n', T:2T] = cos           [N/2, 2T]
  - Device RoPE per pair p, all six tensor_tensor ops on DVE (bf16 2x,
    ~1.21us/op -> ~7.3us/pair paces the fill; GpSimd tensor ops are NOT
    used during the fill - they contend for SBUF ports and stretch DVE ops
    ~4x):  t_es=te*st ; t_os=to*st ; te*=ct ; te-=t_os (QRT_E) ;
           to*=ct ; to+=t_es (QRT_O).
  - Stage 1 (PE): P[s,t] = sum_n QRT[n,s]*QRT[n,t], lower-triangle chains
    packed into 512-col PSUM "units" (34 units).  The kk contraction is
    emitted pair-major and SEGMENTED so the PE keeps working while RoPE
    streams: units 0-3 (banks 0-3, "pp") split A/B at pair 4 and advance
    with the stream; units 4-11 (banks 4-7, "pc") split s1/s2/B at pairs
    2/4 so s1 runs as soon as pairs 0-1 are resident and s2 after pairs
    2-3; units 12-15 split A/B; units 16-19 split at pair 6 (their first
    12 contraction steps fill the late-pair windows); units 20-33 run
    unsplit post-fill.  Segment partials evict to SBUF bf16 (ScalarE);
    s1+s2 partials are recombined by SWDGE DMA-accumulate (touches neither
    DVE nor PE); remaining folds + the strict-causal diagonal masks run on
    GpSimd after the fill.  In a packed unit only the first chain's first
    matmul sets start=True (start clears has_written for the WHOLE bank).
  - Stage 2 (PE): out[t,n] += P[s,t]^T @ V[s,n], accumulating j in PSUM
    ("pp" banks, free after the fill), evict on ScalarE, DMA out.
  - V loads are deferred until after the fill DMA stream.
"""

import os
import sys
import math

for _p in ("/opt/trn_rl_repo", "/root/.axon_site/_ro/trn_rl_repo"):
    if os.path.isdir(_p) and _p not in sys.path:
        sys.path.append(_p)

import numpy as np
import ml_dtypes

B, H, T, N = 2, 4, 2048, 2048
THETA = 2.0 ** 16
NCORES = 8
CW = 512  # superstep width (t-columns) and output n-chunk width

bf16 = ml_dtypes.bfloat16

LAST_RESULT = None  # BassKernelResults of the most recent run (for test.py)


def build_bass(t_len=T, n_dim=N, num_devices=NCORES):
    from concourse import bacc, mybir, tile

    nc = bacc.Bacc("TRN2", target_bir_lowering=False, debug=False,
                   num_devices=num_devices)
    bf = mybir.dt.bfloat16
    f32 = mybir.dt.float32
    mult = mybir.AluOpType.mult
    add = mybir.AluOpType.add

    nh = n_dim // 2
    npair = nh // 128          # RoPE pairs (8); pair p yields qrt[p], qrt[8+p]
    kk_n = n_dim // 128        # contraction tiles (16)
    nb = t_len // 128          # t-blocks (16)
    ncks = t_len // CW         # supersteps (4)
    sw = CW // 128             # t-blocks per superstep (4)
    nch = n_dim // CW          # output n-chunks (4)

    # kk emission order: pair-major so each RoPE pair unlocks two steps.
    kkseq = []
    for p in range(npair):
        kkseq += [p, npair + p]

    qteo = nc.declare_dram_parameter("qteo", [nh, 2 * t_len], bf,
                                     isOutput=False)
    cstd = nc.declare_dram_parameter("cst", [nh, 2 * t_len], bf,
                                     isOutput=False)
    vin = nc.declare_dram_parameter("v", [t_len, n_dim], bf, isOutput=False)
    maskd = nc.declare_dram_parameter("mask", [128, 128], bf, isOutput=False)
    outd = nc.declare_dram_parameter("out", [t_len, n_dim], f32, isOutput=True)

    # ---- stage-1 chain/unit bookkeeping ----
    def chain(ic, j):
        rj0 = max(128 * j, CW * ic)
        return {"ic": ic, "j": j, "rj0": rj0, "w": CW * (ic + 1) - rj0,
                "off": 0}

    ch = {(ic, j): chain(ic, j) for ic in range(ncks)
          for j in range(sw * ic + sw)}

    units = []

    def unit(*keys):
        cs, off = [], 0
        for k in keys:
            c = dict(ch[k])
            c["off"] = off
            off += c["w"]
            cs.append(c)
        assert off <= CW
        units.append({"chains": cs, "w": off, "ps": None, "parts": [],
                      "seg": None, "tag": None, "uid": len(units)})

    unit((0, 0))
    unit((0, 1), (0, 3))
    unit((0, 2), (1, 6))
    unit((1, 0))
    unit((1, 1))
    unit((1, 2))
    unit((1, 3))
    unit((1, 4))
    unit((1, 5), (1, 7))
    for j in range(9):
        unit((2, j))
    unit((2, 9), (2, 11))
    unit((2, 10), (3, 14))
    for j in range(13):
        unit((3, j))
    unit((3, 13), (3, 15))
    assert sum(u["w"] for u in units) == 17408 and len(units) == 34

    SEG2 = [(0, 8), (8, 16)]                    # A (pairs 0-3) / B (4-7)
    SEG3 = [(0, 4), (4, 8), (8, 16)]            # s1 / s2 / B
    SEGL = [(0, 12), (12, 16)]                  # A' (pairs 0-5) / B'
    SEG1 = [(0, 16)]
    # partial-eviction tag per segment class (sized by lifetime)
    for i, u in enumerate(units):
        if i < 4:
            u["seg"], u["tag"] = SEG2, "pp"
        elif i < 12:
            u["seg"], u["tag"] = SEG3, "pc"
        elif i < 16:
            u["seg"], u["tag"] = SEG2, "pc"
        elif i < 20:
            u["seg"], u["tag"] = SEGL, "pc"
        else:
            u["seg"], u["tag"] = SEG1, "pc"

    ptiles = {ic: {} for ic in range(ncks)}  # ic -> j -> (tile, rj0, off)

    with tile.TileContext(nc) as tc:
        with (
            tc.tile_pool(name="qrt", bufs=npair) as qrt_pool,
            tc.tile_pool(name="vres", bufs=nb) as v_pool,
            tc.tile_pool(name="tbl", bufs=2) as tbl_pool,
            tc.tile_pool(name="rtmp", bufs=2) as tmp_pool,
            tc.tile_pool(name="part", bufs=8) as part_pool,
            tc.tile_pool(name="osb", bufs=2) as out_pool,
            tc.tile_pool(name="mk", bufs=1) as mk_pool,
            tc.tile_pool(name="psumP", bufs=4, space="PSUM") as pp_pool,
            tc.tile_pool(name="psumC", bufs=4, space="PSUM") as pc_pool,
        ):
            mask_sb = mk_pool.tile([128, 128], bf)

            qrt = [None] * kk_n
            qtiles, ctiles, ttiles = {}, {}, {}

            def emit_pair(p):
                qp = qrt_pool.tile([128, 2 * t_len], bf, tag="qrt",
                                   name=f"qp_{p}")
                cs = tbl_pool.tile([128, 2 * t_len], bf, tag="tbl",
                                   name=f"cs_{p}")
                rr = slice(128 * p, 128 * p + 128)
                te, to = qp[:, 0:t_len], qp[:, t_len:2 * t_len]
                st, ct = cs[:, 0:t_len], cs[:, t_len:2 * t_len]
                hl = t_len // 2
                if p == 0:
                    # column-split pair 0's DMA so the left t-half lands first
                    for h0 in (0, hl):
                        hs = slice(h0, h0 + hl)
                        nc.sync.dma_start(cs[:, hs], cstd[rr, hs])
                        nc.sync.dma_start(qp[:, hs], qteo[rr, hs])
                        ths = slice(t_len + h0, t_len + h0 + hl)
                        nc.sync.dma_start(qp[:, ths], qteo[rr, ths])
                        nc.sync.dma_start(cs[:, ths], cstd[rr, ths])
                else:
                    nc.sync.dma_start(qp[:], qteo[rr, :])
                    nc.sync.dma_start(cs[:], cstd[rr, :])
                t_es = tmp_pool.tile([128, t_len], bf, tag="tmp",
                                     name=f"tes_{p}")
                t_os = tmp_pool.tile([128, t_len], bf, tag="tmp",
                                     name=f"tos_{p}")
                halves = (slice(0, hl), slice(hl, t_len)) if p == 0 \
                    else (slice(0, t_len),)
                for hs in halves:
                    nc.vector.tensor_mul(t_es[:, hs], te[:, hs], st[:, hs])
                    nc.vector.tensor_mul(t_os[:, hs], to[:, hs], st[:, hs])
                    nc.vector.tensor_mul(te[:, hs], te[:, hs], ct[:, hs])
                    nc.vector.tensor_mul(to[:, hs], to[:, hs], ct[:, hs])
                    nc.vector.tensor_sub(te[:, hs], te[:, hs], t_os[:, hs])
                    nc.vector.tensor_add(to[:, hs], to[:, hs], t_es[:, hs])
                qrt[p] = te
                qrt[npair + p] = to
                qtiles[p], ctiles[p], ttiles[p] = qp, cs, t_es

            # ---- stage-1 machinery ----
            def seg_of(u, kki):
                for s in u["seg"]:
                    if s[0] <= kki < s[1]:
                        return s
                raise AssertionError

            def part_tag(u, s1):
                if s1 == 16:
                    return "pfin", 23
                if u["seg"] is SEG3:
                    return ("ps1", 8) if s1 == 4 else ("ps2", 8)
                return "pA", 12

            def mm_step(u, kki):
                s0, s1 = seg_of(u, kki)
                if kki == s0:
                    pool = pp_pool if u["tag"] == "pp" else pc_pool
                    u["ps"] = pool.tile([128, u["w"]], f32, tag=u["tag"],
                                        name=f"ps_u{u['uid']}_{s0}")
                kk = kkseq[kki]
                for ci, c in enumerate(u["chains"]):
                    # start=True clears has_written for the WHOLE bank, so
                    # only the first chain of a packed unit may set it; the
                    # second chain's first matmul overwrites-where-unset.
                    nc.tensor.matmul(
                        u["ps"][:, c["off"]:c["off"] + c["w"]],
                        qrt[kk][:, 128 * c["j"]:128 * c["j"] + 128],
                        qrt[kk][:, c["rj0"]:c["rj0"] + c["w"]],
                        start=(kki == s0 and ci == 0),
                        stop=(kki == s1 - 1))
                if kki == s1 - 1:
                    tag, bufs = part_tag(u, s1)
                    part = part_pool.tile([128, u["w"]], bf, tag=tag,
                                          bufs=bufs,
                                          name=f"pa_u{u['uid']}_{s0}")
                    nc.scalar.copy(part[:, :], u["ps"][:, :])
                    u["parts"].append(part)
                    u["ps"] = None

            def mm_full(u, k0, k1):
                for kki in range(k0, k1):
                    mm_step(u, kki)

            def finalize(u):
                # folds + masks on DVE: it is idle once the RoPE fill ends,
                # and its tensor_tensor is ~4x faster than GpSimd's.
                parts = [x for x in u["parts"] if x is not None]
                pt = parts[-1]
                for x in parts[:-1]:
                    nc.vector.tensor_add(pt[:, :], pt[:, :], x[:, :])
                for c in u["chains"]:
                    if c["rj0"] == 128 * c["j"]:
                        o = c["off"]
                        nc.vector.tensor_mul(pt[:, o:o + 128],
                                             pt[:, o:o + 128], mask_sb[:])
                    ptiles[c["ic"]][c["j"]] = (pt, c["rj0"], c["off"])

            # ---- stage 2 ----
            v_tiles = [None] * nb

            def stage2(ic):
                pmap = ptiles[ic]
                for d in range(sw):
                    i = sw * ic + d
                    ti = 128 * i
                    for chk in range(nch):
                        ops = pp_pool.tile([128, CW], f32, tag="pp",
                                           name=f"ps2_{i}_{chk}")
                        for j in range(i + 1):
                            pt, rj0, off = pmap[j]
                            o = off + ti - rj0
                            nc.tensor.matmul(
                                ops[:, :], pt[:, o:o + 128],
                                v_tiles[j][:, CW * chk:CW * (chk + 1)],
                                start=(j == 0), stop=(j == i))
                        osb = out_pool.tile([128, CW], f32, tag="osb",
                                            name=f"osb_{i}_{chk}")
                        nc.scalar.copy(osb[:], ops[:])
                        nc.sync.dma_start(
                            outd[ti:ti + 128, CW * chk:CW * (chk + 1)],
                            osb[:])

            # ---- emission schedule ----
            emit_pair(0)
            emit_pair(1)
            # mask DMA off the critical issue path (needed only at finalize)
            nc.sync.dma_start(mask_sb[:], maskd[:])
            for p in range(2, npair):
                emit_pair(p)
            # A: paced window, pairs 0-1: units 0-7 advance with the stream.
            for kki in range(0, 4):
                for u in units[0:8]:
                    mm_step(u, kki)
            # B: units 8-11 s1 full-speed on resident pairs 0-1.
            for u in units[8:12]:
                mm_full(u, 0, 4)
            # C: pairs 2-3: units 0-3 continue A, units 4-7 s2.
            for kki in range(4, 8):
                for u in units[0:8]:
                    mm_step(u, kki)
            # D: units 8-11 s2 full-speed (s1+s2 fold happens in finalize,
            # on the post-fill-idle DVE).
            for u in units[8:12]:
                mm_full(u, 4, 8)
            # E: units 12-15 A full-speed on resident pairs 0-3.
            for u in units[12:16]:
                mm_full(u, 0, 8)
            # F/G/H/I/J: pairs 4-7 windows: units 0-3 B paced, units 16-19
            # A' (12 steps) filling the gaps.
            for kki in range(8, 10):
                for u in units[0:4]:
                    mm_step(u, kki)
            for u in units[16:20]:
                mm_full(u, 0, 12)
            for kki in range(10, 12):
                for u in units[0:4]:
                    mm_step(u, kki)
            for u in units[4:8]:
                mm_full(u, 8, 12)
            for kki in range(12, 16):
                for u in units[0:8]:
                    mm_step(u, kki)
            for u in units[0:8]:
                finalize(u)
            # V loads: issue after the whole fill DMA stream.
            for jb in range(nb):
                vt = v_pool.tile([128, n_dim], bf, tag="vt",
                                 name=f"vt_{jb}")
                nc.sync.dma_start(vt[:], vin[128 * jb:128 * (jb + 1), :])
                v_tiles[jb] = vt
            # F4: B segments + unsplit chains, interleaved with stage 2.
            # stage2(1) only needs ic1 ptiles (units 2-8), so it runs right
            # after stage2(0) while GpSimd folds units 9-19 in parallel.
            mm_full(units[8], 8, 16)
            finalize(units[8])
            stage2(0)
            stage2(1)
            for u in units[9:16]:
                mm_full(u, 8, 16)
                finalize(u)
            for u in units[16:20]:
                mm_full(u, 12, 16)
                finalize(u)
            for u in units[20:27]:
                mm_full(u, 0, 16)
                finalize(u)
            stage2(2)
            for u in units[27:34]:
                mm_full(u, 0, 16)
                finalize(u)
            stage2(3)

    nc.compile()
    return nc


def _tables(t_len=T, n_dim=N):
    t = np.arange(n_dim, dtype=np.float32)
    q = np.floor(t / 2.0) * 2.0
    f = (1.0 / THETA ** (q.astype(np.float64) / n_dim)
         / (2.0 * math.pi)).astype(np.float32)
    phases = np.arange(t_len, dtype=np.float32)[:, None] * f[None, :]
    ph = (phases % 1.0) * np.float32(2.0 * math.pi)
    ct = np.ascontiguousarray(np.cos(ph)[:, 0::2].T)  # [N/2, T]
    st = np.ascontiguousarray(np.sin(ph)[:, 0::2].T)
    return np.concatenate([st, ct], axis=1).astype(bf16)  # [N/2, 2T]


def _mask128():
    s = np.arange(128)[:, None]
    tt = np.arange(128)[None, :]
    return (s < tt).astype(bf16)


_compiled = {}


def _get_nc():
    if "nc" not in _compiled:
        _compiled["nc"] = build_bass()
    return _compiled["nc"]


def kernel(Q, V):
    global LAST_RESULT
    from concourse.bass_utils import run_bass_kernel_spmd

    Q = np.asarray(Q)
    V = np.asarray(V)
    assert Q.shape == (B, H, T, N) and V.shape == (B, H, T, N)

    nc = _get_nc()
    cst = _tables()
    mask = _mask128()

    in_maps = []
    for b in range(B):
        for h in range(H):
            qs = Q[b, h]
            qteo = np.concatenate(
                [qs[:, 0::2].T, qs[:, 1::2].T], axis=1)  # [N/2, 2T]
            in_maps.append({
                "qteo": np.ascontiguousarray(qteo).astype(bf16),
                "cst": cst,
                "v": V[b, h].astype(bf16),
                "mask": mask,
            })

    res = run_bass_kernel_spmd(nc, in_maps, core_ids=list(range(NCORES)))
    LAST_RESULT = res

    out = np.empty((B, H, T, N), dtype=np.float32)
    for b in range(B):
        for h in range(H):
            out[b, h] = res.results[b * H + h]["out"]
    return out


# revision 22
# speedup vs baseline: 1.1951x; 1.0044x over previous
"""Trainium2 Bass kernel for strictly-causal RoPE self-attention (no softmax).

  out[b,h] = tril(rope(Q)@rope(Q)^T, -1) @ V    with K = Q.

Sharding: B*H = 8 independent (b,h) slices -> one per NeuronCore (pure data
parallel, no collectives). Per core: T=N=2048.

Per-core algorithm (device compute in bf16 matmul / f32 accumulate):
  - Host passes Q pre-transposed+deinterleaved (layout prep only), with the
    even/odd planes and the sin/cos tables CONCATENATED column-wise so each
    RoPE pair needs just two 1 MB DMAs (the in-order Sync queue serializes
    DMA issues, so issue count is precious):
      qteo[n', 0:T] = Q[t,2n'], qteo[n', T:2T] = Q[t,2n'+1]    [N/2, 2T]
      cst [n', 0:T] = sin,      cst [n', T:2T] = cos           [N/2, 2T]
  - Device RoPE per pair p, all six tensor_tensor ops on DVE (bf16 2x,
    ~1.21us/op -> ~7.3us/pair paces the fill; GpSimd tensor ops are NOT
    used during the fill - they contend for SBUF ports and stretch DVE ops
    ~4x):  t_es=te*st ; t_os=to*st ; te*=ct ; te-=t_os (QRT_E) ;
           to*=ct ; to+=t_es (QRT_O).
  - Stage 1 (PE): P[s,t] = sum_n QRT[n,s]*QRT[n,t], lower-triangle chains
    packed into 512-col PSUM "units" (34 units).  The kk contraction is
    emitted pair-major and SEGMENTED so the PE keeps working while RoPE
    streams: units 0-3 (banks 0-3, "pp") split A/B at pair 4 and advance
    with the stream; units 4-11 (banks 4-7, "pc") split s1/s2/B at pairs
    2/4 so s1 runs as soon as pairs 0-1 are resident and s2 after pairs
    2-3; units 12-15 split A/B; units 16-19 split at pair 6 (their first
    12 contraction steps fill the late-pair windows); units 20-33 run
    unsplit post-fill.  Segment partials evict to SBUF bf16 (ScalarE);
    s1+s2 partials are recombined by SWDGE DMA-accumulate (touches neither
    DVE nor PE); remaining folds + the strict-causal diagonal masks run on
    GpSimd after the fill.  In a packed unit only the first chain's first
    matmul sets start=True (start clears has_written for the WHOLE bank).
  - Stage 2 (PE): out[t,n] += P[s,t]^T @ V[s,n], accumulating j in PSUM
    ("pp" banks, free after the fill), evict on ScalarE, DMA out.
  - V loads are deferred until after the fill DMA stream.
"""

import os
import sys
import math

for _p in ("/opt/trn_rl_repo", "/root/.axon_site/_ro/trn_rl_repo"):
    if os.path.isdir(_p) and _p not in sys.path:
        sys.path.append(_p)

import numpy as np
import ml_dtypes

B, H, T, N = 2, 4, 2048, 2048
THETA = 2.0 ** 16
NCORES = 8
CW = 512  # superstep width (t-columns) and output n-chunk width

bf16 = ml_dtypes.bfloat16

LAST_RESULT = None  # BassKernelResults of the most recent run (for test.py)


def build_bass(t_len=T, n_dim=N, num_devices=NCORES):
    from concourse import bacc, mybir, tile

    nc = bacc.Bacc("TRN2", target_bir_lowering=False, debug=False,
                   num_devices=num_devices)
    bf = mybir.dt.bfloat16
    f32 = mybir.dt.float32
    mult = mybir.AluOpType.mult
    add = mybir.AluOpType.add

    nh = n_dim // 2
    npair = nh // 128          # RoPE pairs (8); pair p yields qrt[p], qrt[8+p]
    kk_n = n_dim // 128        # contraction tiles (16)
    nb = t_len // 128          # t-blocks (16)
    ncks = t_len // CW         # supersteps (4)
    sw = CW // 128             # t-blocks per superstep (4)
    nch = n_dim // CW          # output n-chunks (4)

    # kk emission order: pair-major so each RoPE pair unlocks two steps.
    kkseq = []
    for p in range(npair):
        kkseq += [p, npair + p]

    qteo = nc.declare_dram_parameter("qteo", [nh, 2 * t_len], bf,
                                     isOutput=False)
    cstd = nc.declare_dram_parameter("cst", [nh, 2 * t_len], bf,
                                     isOutput=False)
    vin = nc.declare_dram_parameter("v", [t_len, n_dim], bf, isOutput=False)
    maskd = nc.declare_dram_parameter("mask", [128, 128], bf, isOutput=False)
    outd = nc.declare_dram_parameter("out", [t_len, n_dim], f32, isOutput=True)

    # ---- stage-1 chain/unit bookkeeping ----
    def chain(ic, j):
        rj0 = max(128 * j, CW * ic)
        return {"ic": ic, "j": j, "rj0": rj0, "w": CW * (ic + 1) - rj0,
                "off": 0}

    ch = {(ic, j): chain(ic, j) for ic in range(ncks)
          for j in range(sw * ic + sw)}

    units = []

    def unit(*keys):
        cs, off = [], 0
        for k in keys:
            c = dict(ch[k])
            c["off"] = off
            off += c["w"]
            cs.append(c)
        assert off <= CW
        units.append({"chains": cs, "w": off, "ps": None, "parts": [],
                      "seg": None, "tag": None, "uid": len(units)})

    unit((0, 0))
    unit((0, 1), (0, 3))
    unit((0, 2), (1, 6))
    unit((1, 0))
    unit((1, 1))
    unit((1, 2))
    unit((1, 3))
    unit((1, 4))
    unit((1, 5), (1, 7))
    for j in range(9):
        unit((2, j))
    unit((2, 9), (2, 11))
    unit((2, 10), (3, 14))
    for j in range(13):
        unit((3, j))
    unit((3, 13), (3, 15))
    assert sum(u["w"] for u in units) == 17408 and len(units) == 34

    SEG2 = [(0, 8), (8, 16)]                    # A (pairs 0-3) / B (4-7)
    SEG3 = [(0, 4), (4, 8), (8, 16)]            # s1 / s2 / B
    SEGL = [(0, 12), (12, 16)]                  # A' (pairs 0-5) / B'
    SEG1 = [(0, 16)]
    # partial-eviction tag per segment class (sized by lifetime)
    for i, u in enumerate(units):
        if i < 4:
            u["seg"], u["tag"] = SEG2, "pp"
        elif i < 12:
            u["seg"], u["tag"] = SEG3, "pc"
        elif i < 16:
            u["seg"], u["tag"] = SEG2, "pc"
        elif i < 20:
            u["seg"], u["tag"] = SEGL, "pc"
        else:
            u["seg"], u["tag"] = SEG1, "pc"

    ptiles = {ic: {} for ic in range(ncks)}  # ic -> j -> (tile, rj0, off)

    with tile.TileContext(nc) as tc:
        with (
            tc.tile_pool(name="qrt", bufs=npair) as qrt_pool,
            tc.tile_pool(name="vres", bufs=nb) as v_pool,
            tc.tile_pool(name="tbl", bufs=2) as tbl_pool,
            tc.tile_pool(name="rtmp", bufs=2) as tmp_pool,
            tc.tile_pool(name="part", bufs=8) as part_pool,
            tc.tile_pool(name="osb", bufs=2) as out_pool,
            tc.tile_pool(name="mk", bufs=1) as mk_pool,
            tc.tile_pool(name="psumP", bufs=4, space="PSUM") as pp_pool,
            tc.tile_pool(name="psumC", bufs=4, space="PSUM") as pc_pool,
        ):
            mask_sb = mk_pool.tile([128, 128], bf)

            qrt = [None] * kk_n
            qtiles, ctiles, ttiles = {}, {}, {}

            def emit_pair(p):
                qp = qrt_pool.tile([128, 2 * t_len], bf, tag="qrt",
                                   name=f"qp_{p}")
                cs = tbl_pool.tile([128, 2 * t_len], bf, tag="tbl",
                                   name=f"cs_{p}")
                rr = slice(128 * p, 128 * p + 128)
                te, to = qp[:, 0:t_len], qp[:, t_len:2 * t_len]
                st, ct = cs[:, 0:t_len], cs[:, t_len:2 * t_len]
                hl = t_len // 2
                if p == 0:
                    # column-split pair 0's DMA so the left t-half lands first
                    for h0 in (0, hl):
                        hs = slice(h0, h0 + hl)
                        nc.sync.dma_start(cs[:, hs], cstd[rr, hs])
                        nc.sync.dma_start(qp[:, hs], qteo[rr, hs])
                        ths = slice(t_len + h0, t_len + h0 + hl)
                        nc.sync.dma_start(qp[:, ths], qteo[rr, ths])
                        nc.sync.dma_start(cs[:, ths], cstd[rr, ths])
                else:
                    nc.sync.dma_start(qp[:], qteo[rr, :])
                    nc.sync.dma_start(cs[:], cstd[rr, :])
                t_es = tmp_pool.tile([128, t_len], bf, tag="tmp",
                                     name=f"tes_{p}")
                t_os = tmp_pool.tile([128, t_len], bf, tag="tmp",
                                     name=f"tos_{p}")
                halves = (slice(0, hl), slice(hl, t_len))
                for hs in halves:
                    nc.vector.tensor_mul(t_es[:, hs], te[:, hs], st[:, hs])
                    nc.vector.tensor_mul(t_os[:, hs], to[:, hs], st[:, hs])
                    nc.vector.tensor_mul(te[:, hs], te[:, hs], ct[:, hs])
                    nc.vector.tensor_mul(to[:, hs], to[:, hs], ct[:, hs])
                    nc.vector.tensor_sub(te[:, hs], te[:, hs], t_os[:, hs])
                    nc.vector.tensor_add(to[:, hs], to[:, hs], t_es[:, hs])
                qrt[p] = te
                qrt[npair + p] = to
                qtiles[p], ctiles[p], ttiles[p] = qp, cs, t_es

            # ---- stage-1 machinery ----
            def seg_of(u, kki):
                for s in u["seg"]:
                    if s[0] <= kki < s[1]:
                        return s
                raise AssertionError

            def part_tag(u, s1):
                if s1 == 16:
                    return "pfin", 23
                if u["seg"] is SEG3:
                    return ("ps1", 8) if s1 == 4 else ("ps2", 8)
                return "pA", 12

            def mm_step(u, kki):
                s0, s1 = seg_of(u, kki)
                if kki == s0:
                    pool = pp_pool if u["tag"] == "pp" else pc_pool
                    u["ps"] = pool.tile([128, u["w"]], f32, tag=u["tag"],
                                        name=f"ps_u{u['uid']}_{s0}")
                kk = kkseq[kki]
                for ci, c in enumerate(u["chains"]):
                    # start=True clears has_written for the WHOLE bank, so
                    # only the first chain of a packed unit may set it; the
                    # second chain's first matmul overwrites-where-unset.
                    nc.tensor.matmul(
                        u["ps"][:, c["off"]:c["off"] + c["w"]],
                        qrt[kk][:, 128 * c["j"]:128 * c["j"] + 128],
                        qrt[kk][:, c["rj0"]:c["rj0"] + c["w"]],
                        start=(kki == s0 and ci == 0),
                        stop=(kki == s1 - 1))
                if kki == s1 - 1:
                    tag, bufs = part_tag(u, s1)
                    part = part_pool.tile([128, u["w"]], bf, tag=tag,
                                          bufs=bufs,
                                          name=f"pa_u{u['uid']}_{s0}")
                    nc.scalar.copy(part[:, :], u["ps"][:, :])
                    u["parts"].append(part)
                    u["ps"] = None

            def mm_full(u, k0, k1):
                for kki in range(k0, k1):
                    mm_step(u, kki)

            def finalize(u):
                # folds + masks on DVE: it is idle once the RoPE fill ends,
                # and its tensor_tensor is ~4x faster than GpSimd's.
                parts = [x for x in u["parts"] if x is not None]
                pt = parts[-1]
                for x in parts[:-1]:
                    nc.vector.tensor_add(pt[:, :], pt[:, :], x[:, :])
                for c in u["chains"]:
                    if c["rj0"] == 128 * c["j"]:
                        o = c["off"]
                        nc.vector.tensor_mul(pt[:, o:o + 128],
                                             pt[:, o:o + 128], mask_sb[:])
                    ptiles[c["ic"]][c["j"]] = (pt, c["rj0"], c["off"])

            # ---- stage 2 ----
            v_tiles = [None] * nb

            def stage2(ic):
                pmap = ptiles[ic]
                for d in range(sw):
                    i = sw * ic + d
                    ti = 128 * i
                    for chk in range(nch):
                        ops = pp_pool.tile([128, CW], f32, tag="pp",
                                           name=f"ps2_{i}_{chk}")
                        for j in range(i + 1):
                            pt, rj0, off = pmap[j]
                            o = off + ti - rj0
                            nc.tensor.matmul(
                                ops[:, :], pt[:, o:o + 128],
                                v_tiles[j][:, CW * chk:CW * (chk + 1)],
                                start=(j == 0), stop=(j == i))
                        osb = out_pool.tile([128, CW], f32, tag="osb",
                                            name=f"osb_{i}_{chk}")
                        nc.scalar.copy(osb[:], ops[:])
                        nc.sync.dma_start(
                            outd[ti:ti + 128, CW * chk:CW * (chk + 1)],
                            osb[:])

            # ---- emission schedule ----
            emit_pair(0)
            emit_pair(1)
            # mask DMA off the critical issue path (needed only at finalize)
            nc.sync.dma_start(mask_sb[:], maskd[:])
            for p in range(2, npair):
                emit_pair(p)
            # A: paced window, pairs 0-1: units 0-7 advance with the stream.
            for kki in range(0, 4):
                for u in units[0:8]:
                    mm_step(u, kki)
            # B: units 8-11 s1 full-speed on resident pairs 0-1.
            for u in units[8:12]:
                mm_full(u, 0, 4)
            # C: pairs 2-3: units 0-3 continue A, units 4-7 s2.
            for kki in range(4, 8):
                for u in units[0:8]:
                    mm_step(u, kki)
            # D: units 8-11 s2 full-speed (s1+s2 fold happens in finalize,
            # on the post-fill-idle DVE).
            for u in units[8:12]:
                mm_full(u, 4, 8)
            # E: units 12-15 A full-speed on resident pairs 0-3.
            for u in units[12:16]:
                mm_full(u, 0, 8)
            # F/G/H/I/J: pairs 4-7 windows: units 0-3 B paced, units 16-19
            # A' (12 steps) filling the gaps.
            for kki in range(8, 10):
                for u in units[0:4]:
                    mm_step(u, kki)
            for u in units[16:20]:
                mm_full(u, 0, 12)
            for kki in range(10, 12):
                for u in units[0:4]:
                    mm_step(u, kki)
            for u in units[4:8]:
                mm_full(u, 8, 12)
            for kki in range(12, 16):
                for u in units[0:8]:
                    mm_step(u, kki)
            for u in units[0:8]:
                finalize(u)
            # V loads: issue after the whole fill DMA stream.
            for jb in range(nb):
                vt = v_pool.tile([128, n_dim], bf, tag="vt",
                                 name=f"vt_{jb}")
                nc.sync.dma_start(vt[:], vin[128 * jb:128 * (jb + 1), :])
                v_tiles[jb] = vt
            # F4: B segments + unsplit chains, interleaved with stage 2.
            # stage2(1) only needs ic1 ptiles (units 2-8), so it runs right
            # after stage2(0) while GpSimd folds units 9-19 in parallel.
            mm_full(units[8], 8, 16)
            finalize(units[8])
            stage2(0)
            stage2(1)
            for u in units[9:16]:
                mm_full(u, 8, 16)
                finalize(u)
            for u in units[16:20]:
                mm_full(u, 12, 16)
                finalize(u)
            for u in units[20:27]:
                mm_full(u, 0, 16)
                finalize(u)
            stage2(2)
            for u in units[27:34]:
                mm_full(u, 0, 16)
                finalize(u)
            stage2(3)

    nc.compile()
    return nc


def _tables(t_len=T, n_dim=N):
    t = np.arange(n_dim, dtype=np.float32)
    q = np.floor(t / 2.0) * 2.0
    f = (1.0 / THETA ** (q.astype(np.float64) / n_dim)
         / (2.0 * math.pi)).astype(np.float32)
    phases = np.arange(t_len, dtype=np.float32)[:, None] * f[None, :]
    ph = (phases % 1.0) * np.float32(2.0 * math.pi)
    ct = np.ascontiguousarray(np.cos(ph)[:, 0::2].T)  # [N/2, T]
    st = np.ascontiguousarray(np.sin(ph)[:, 0::2].T)
    return np.concatenate([st, ct], axis=1).astype(bf16)  # [N/2, 2T]


def _mask128():
    s = np.arange(128)[:, None]
    tt = np.arange(128)[None, :]
    return (s < tt).astype(bf16)


_compiled = {}


def _get_nc():
    if "nc" not in _compiled:
        _compiled["nc"] = build_bass()
    return _compiled["nc"]


def kernel(Q, V):
    global LAST_RESULT
    from concourse.bass_utils import run_bass_kernel_spmd

    Q = np.asarray(Q)
    V = np.asarray(V)
    assert Q.shape == (B, H, T, N) and V.shape == (B, H, T, N)

    nc = _get_nc()
    cst = _tables()
    mask = _mask128()

    in_maps = []
    for b in range(B):
        for h in range(H):
            qs = Q[b, h]
            qteo = np.concatenate(
                [qs[:, 0::2].T, qs[:, 1::2].T], axis=1)  # [N/2, 2T]
            in_maps.append({
                "qteo": np.ascontiguousarray(qteo).astype(bf16),
                "cst": cst,
                "v": V[b, h].astype(bf16),
                "mask": mask,
            })

    res = run_bass_kernel_spmd(nc, in_maps, core_ids=list(range(NCORES)))
    LAST_RESULT = res

    out = np.empty((B, H, T, N), dtype=np.float32)
    for b in range(B):
        for h in range(H):
            out[b, h] = res.results[b * H + h]["out"]
    return out
